# revision 6
# baseline (speedup 1.0000x reference)
"""Inverse 2D Haar DWT (idwt2) Trainium2 Bass kernel.

Full inputs: approximation/detail_h/detail_v/detail_d each [8, 64, 128, 128] f32.
Full output: [8, 64, 256, 256] f32 with out 2x2 blocks:
  x00 = (a + v + h + d)/2   at [2i,   2j]
  x01 = (a - v + h - d)/2   at [2i,   2j+1]
  x10 = (a + v - h - d)/2   at [2i+1, 2j]
  x11 = (a - v - h + d)/2   at [2i+1, 2j+1]

Sharding: batch dim across 8 cores (1 batch each), no communication.

Per-core layout trick: view the (64,128,128) input as [128, 8192] where
partition P = 2*c + (i>=64) holds rows i in [64*(P%2), 64*(P%2)+64) of
channel c = P//2, each partition's data fully contiguous in DRAM. The
(64,256,256) output viewed as [128, 32768] has the *same* partition map
(P = 2*c + (i2>=128)), so input loads and output stores are both fully
contiguous DMAs with multi-KB descriptors.

Butterfly: with p=(a+h)/2, r=(a-h)/2, q=(v+d)/2, s=(v-d)/2:
  x00=p+q, x01=p-q, x10=r+s, x11=r-s.
Tiles pack [a|v] and [h|d] side by side so one ACT op prescales both
(avs = [a|v]/2), one STT computes [p|q] and one computes [r|s], and two
final TT ops (add/sub over 4-D strided APs) write x00/x10 and x01/x11
straight into the interleaved row-pair-packed output tile, keeping the
store DMA linear. Loads ride the SP HWDGE ring, stores the ACT ring, so
stores never head-of-line block loads.
"""

import numpy as np

B, C, H, W = 8, 64, 128, 128
N_CORES = 8
R = 8  # rows (of 64 per partition block) processed per group
G = 64 // R

_cache = {}


def _build():
    import concourse.bacc as bacc
    import concourse.tile as tile
    from concourse import mybir

    fp32 = mybir.dt.float32
    add = mybir.AluOpType.add
    sub = mybir.AluOpType.subtract
    mult = mybir.AluOpType.mult

    nc = bacc.Bacc("TRN2", target_bir_lowering=False, debug=False)

    names = ["approximation", "detail_h", "detail_v", "detail_d"]
    ins = {
        n: nc.dram_tensor(n, [128, 64 * 128], fp32, kind="ExternalInput").ap()
        for n in names
    }
    out = nc.dram_tensor("out", [128, 128 * 256], fp32, kind="ExternalOutput").ap()

    FD = R * 128  # free-dim elems per input tile per tensor

    with tile.TileContext(nc) as tc:
        with (
            tc.tile_pool(name="inp", bufs=4) as inp,
            tc.tile_pool(name="tmp", bufs=2) as tmp,
            tc.tile_pool(name="outp", bufs=3) as outp,
        ):
            for g in range(G):
                isl = slice(g * FD, (g + 1) * FD)
                av = inp.tile([128, 2 * FD], fp32, tag="av")
                hd = inp.tile([128, 2 * FD], fp32, tag="hd")
                nc.sync.dma_start(out=av[:, 0:FD], in_=ins["approximation"][:, isl])
                nc.sync.dma_start(out=hd[:, 0:FD], in_=ins["detail_h"][:, isl])
                nc.sync.dma_start(out=av[:, FD : 2 * FD], in_=ins["detail_v"][:, isl])
                nc.sync.dma_start(out=hd[:, FD : 2 * FD], in_=ins["detail_d"][:, isl])

                avs = tmp.tile([128, 2 * FD], fp32, tag="avs")
                nc.scalar.mul(avs[:], av[:], 0.5)  # [a|v]/2

                pqrs = tmp.tile([128, 4 * FD], fp32, tag="pqrs")
                # [p|q] = ([h|d]*0.5) + [a|v]/2 ; [r|s] = ([h|d]*-0.5) + [a|v]/2
                nc.vector.scalar_tensor_tensor(
                    pqrs[:, 0 : 2 * FD], hd[:], 0.5, avs[:], mult, add
                )
                nc.vector.scalar_tensor_tensor(
                    pqrs[:, 2 * FD : 4 * FD], hd[:], -0.5, avs[:], mult, add
                )

                to = outp.tile([128, R * 512], fp32, tag="o")
                # {p,r} and {q,s} as [128, 2, R, 128] strided views
                v4 = pqrs[:].rearrange("p (t f) -> p t f", t=2)
                in0 = v4[:, :, 0:FD].rearrange("p t (r w) -> p t r w", w=128)
                in1 = v4[:, :, FD : 2 * FD].rearrange("p t (r w) -> p t r w", w=128)
                # output rows packed [top(256) | bot(256)] per input row:
                # t selects top/bot half, inner 256 sliced by 2 interleaves
                o4 = to[:].rearrange("p (r t x) -> p t r x", t=2, x=256)
                nc.vector.tensor_tensor(o4[:, :, :, 0:256:2], in0, in1, add)
                nc.vector.tensor_tensor(o4[:, :, :, 1:256:2], in0, in1, sub)

                osl = slice(g * R * 512, (g + 1) * R * 512)
                if g == G - 1:
                    # split the final store across both rings: halves drain time
                    m = g * R * 512 + R * 256
                    nc.scalar.dma_start(out=out[:, osl.start : m], in_=to[:, : R * 256])
                    nc.sync.dma_start(out=out[:, m : osl.stop], in_=to[:, R * 256 :])
                else:
                    nc.scalar.dma_start(out=out[:, osl], in_=to[:])

    nc.compile()
    return nc


def kernel(approximation, detail_h, detail_v, detail_d):
    from concourse.bass_utils import run_bass_kernel_spmd

    if "nc" not in _cache:
        _cache["nc"] = _build()
    nc = _cache["nc"]

    full = {
        "approximation": approximation,
        "detail_h": detail_h,
        "detail_v": detail_v,
        "detail_d": detail_d,
    }
    in_maps = [
        {
            k: np.ascontiguousarray(v[b]).reshape(128, 64 * 128)
            for k, v in full.items()
        }
        for b in range(N_CORES)
    ]
    res = run_bass_kernel_spmd(nc, in_maps, list(range(N_CORES)))
    out = np.stack(
        [res.results[b]["out"].reshape(C, 2 * H, 2 * W) for b in range(N_CORES)]
    )
    return out.astype(np.float32, copy=False)


# revision 7
# speedup vs baseline: 1.0733x; 1.0733x over previous
"""Inverse 2D Haar DWT (idwt2) Trainium2 Bass kernel.

Full inputs: approximation/detail_h/detail_v/detail_d each [8, 64, 128, 128] f32.
Full output: [8, 64, 256, 256] f32 with out 2x2 blocks:
  x00 = (a + v + h + d)/2   at [2i,   2j]
  x01 = (a - v + h - d)/2   at [2i,   2j+1]
  x10 = (a + v - h - d)/2   at [2i+1, 2j]
  x11 = (a - v - h + d)/2   at [2i+1, 2j+1]

Sharding: batch dim across 8 cores (1 batch each), no communication.

Per-core layout trick: view the (64,128,128) input as [128, 8192] where
partition P = 2*c + (i>=64) holds rows i in [64*(P%2), 64*(P%2)+64) of
channel c = P//2, each partition's data fully contiguous in DRAM. The
(64,256,256) output viewed as [128, 32768] has the *same* partition map
(P = 2*c + (i2>=128)), so input loads and output stores are both fully
contiguous DMAs with multi-KB descriptors.

Butterfly: with p=(a+h)/2, r=(a-h)/2, q=(v+d)/2, s=(v-d)/2:
  x00=p+q, x01=p-q, x10=r+s, x11=r-s.
Tiles pack [a|v] and [h|d] side by side so one ACT op prescales both
(avs = [a|v]/2), one STT computes [p|q] and one computes [r|s], and two
final TT ops (add/sub over 4-D strided APs) write x00/x10 and x01/x11
straight into the interleaved row-pair-packed output tile, keeping the
store DMA linear. Loads ride the SP HWDGE ring, stores the ACT ring, so
stores never head-of-line block loads.
"""

import numpy as np

B, C, H, W = 8, 64, 128, 128
N_CORES = 8
R = 8  # rows (of 64 per partition block) processed per group
G = 64 // R

_cache = {}


def _build():
    import concourse.bacc as bacc
    import concourse.tile as tile
    from concourse import mybir

    fp32 = mybir.dt.float32
    add = mybir.AluOpType.add
    sub = mybir.AluOpType.subtract
    mult = mybir.AluOpType.mult

    nc = bacc.Bacc("TRN2", target_bir_lowering=False, debug=False)

    names = ["approximation", "detail_h", "detail_v", "detail_d"]
    ins = {
        n: nc.dram_tensor(n, [128, 64 * 128], fp32, kind="ExternalInput").ap()
        for n in names
    }
    out = nc.dram_tensor("out", [128, 128 * 256], fp32, kind="ExternalOutput").ap()

    # row-block sizes per group: split the last R-block in half so the
    # final store (which can't overlap anything) is half as large
    blocks = [R] * (G - 1) + [R // 2, R // 2]

    with tile.TileContext(nc) as tc:
        with (
            tc.tile_pool(name="inp", bufs=5) as inp,
            tc.tile_pool(name="tmp", bufs=2) as tmp,
            tc.tile_pool(name="outp", bufs=3) as outp,
        ):
            r0 = 0
            for gi, rb in enumerate(blocks):
                FD = rb * 128
                isl = slice(r0 * 128, (r0 + rb) * 128)
                av = inp.tile([128, 2 * FD], fp32, tag="av")
                hd = inp.tile([128, 2 * FD], fp32, tag="hd")
                nc.sync.dma_start(out=av[:, 0:FD], in_=ins["approximation"][:, isl])
                nc.sync.dma_start(out=hd[:, 0:FD], in_=ins["detail_h"][:, isl])
                nc.sync.dma_start(out=av[:, FD : 2 * FD], in_=ins["detail_v"][:, isl])
                nc.sync.dma_start(out=hd[:, FD : 2 * FD], in_=ins["detail_d"][:, isl])

                avs = tmp.tile([128, 2 * FD], fp32, tag="avs")
                nc.scalar.mul(avs[:], av[:], 0.5)  # [a|v]/2

                pqrs = tmp.tile([128, 4 * FD], fp32, tag="pqrs")
                # [p|q] = ([h|d]*0.5) + [a|v]/2 ; [r|s] = ([h|d]*-0.5) + [a|v]/2
                nc.vector.scalar_tensor_tensor(
                    pqrs[:, 0 : 2 * FD], hd[:], 0.5, avs[:], mult, add
                )
                nc.vector.scalar_tensor_tensor(
                    pqrs[:, 2 * FD : 4 * FD], hd[:], -0.5, avs[:], mult, add
                )

                to = outp.tile([128, rb * 512], fp32, tag="o")
                # {p,r} and {q,s} as [128, 2, rb, 128] strided views
                v4 = pqrs[:].rearrange("p (t f) -> p t f", t=2)
                in0 = v4[:, :, 0:FD].rearrange("p t (r w) -> p t r w", w=128)
                in1 = v4[:, :, FD : 2 * FD].rearrange("p t (r w) -> p t r w", w=128)
                # output rows packed [top(256) | bot(256)] per input row:
                # t selects top/bot half, inner 256 sliced by 2 interleaves
                o4 = to[:].rearrange("p (r t x) -> p t r x", t=2, x=256)
                nc.vector.tensor_tensor(o4[:, :, :, 0:256:2], in0, in1, add)
                nc.vector.tensor_tensor(o4[:, :, :, 1:256:2], in0, in1, sub)

                osl = slice(r0 * 512, (r0 + rb) * 512)
                if gi == len(blocks) - 1:
                    # split the final store across both rings: halves drain time
                    m = (r0 + rb // 2) * 512
                    nc.scalar.dma_start(
                        out=out[:, osl.start : m], in_=to[:, : (rb // 2) * 512]
                    )
                    nc.sync.dma_start(
                        out=out[:, m : osl.stop], in_=to[:, (rb // 2) * 512 :]
                    )
                else:
                    nc.scalar.dma_start(out=out[:, osl], in_=to[:])
                r0 += rb

    nc.compile()
    return nc


def kernel(approximation, detail_h, detail_v, detail_d):
    from concourse.bass_utils import run_bass_kernel_spmd

    if "nc" not in _cache:
        _cache["nc"] = _build()
    nc = _cache["nc"]

    full = {
        "approximation": approximation,
        "detail_h": detail_h,
        "detail_v": detail_v,
        "detail_d": detail_d,
    }
    in_maps = [
        {
            k: np.ascontiguousarray(v[b]).reshape(128, 64 * 128)
            for k, v in full.items()
        }
        for b in range(N_CORES)
    ]
    res = run_bass_kernel_spmd(nc, in_maps, list(range(N_CORES)))
    out = np.stack(
        [res.results[b]["out"].reshape(C, 2 * H, 2 * W) for b in range(N_CORES)]
    )
    return out.astype(np.float32, copy=False)
